# revision 2
# baseline (speedup 1.0000x reference)
"""Trainium2 Bass kernel for nn_ComplexNet: out = x @ M_r.T

Reference math: x_imag = 0, so only M_r (the real coefficient matrix,
[2, 10], built from psi/A via a tiny einsum) matters:
    out[t, k] = sum_a x[t, a] * M_r[k, a]

The problem is pure memory streaming; the only lever that matters is HBM
bytes moved.  The rel-err budget (2e-2) is ~100x looser than fp16
round-off, so the host stages x in fp16 (and reads the result back in
fp16): 12.3 MB/core instead of 24.6 MB/core -> ~34 us DMA floor at the
360 GB/s per-core bus, vs 68 us for f32.

Layout (chosen so the device does zero data rearrangement):
  - Host casts + transposes x into a per-core [128, 5, F] fp16 tensor:
    partition p = 2g+c (g = row-group 0..63, c = pair lane), column
    (j, f) holds x[row = f*64 + g, feature = 2j+c].
  - Stationary weights W_j [128, 128] (block 2x2 diagonal):
    W_j[2g+c, 2g+k] = M[k, 2j+c].  One fp16 matmul per (tile, j),
    j = 0..4 accumulating in PSUM:
      psum[2g+k, f] = sum_{j,c} x[f*64+g, 2j+c] * M[k, 2j+c]
    i.e. 64 rows x 2 outputs per moving column - 12.8 rows/cycle at the
    warm 2.4 GHz PE clock (~18 us/core, under the DMA floor).
  - ACT evicts each [128, 512] PSUM bank to SBUF as fp16 (~10 us/core),
    gpsimd (SWDGE) DMAs it out; host un-permutes and upcasts.

kernel(**inputs) takes the FULL unsharded inputs, returns the FULL
[4_000_000, 2] float32 output.
"""

import sys

import numpy as np

if "/opt/trn_rl_repo" not in sys.path:
    sys.path.insert(0, "/opt/trn_rl_repo")

from contextlib import ExitStack

import concourse.bacc as bacc
import concourse.tile as tile
from concourse import mybir
from concourse.bass_utils import run_bass_kernel_spmd

T = 4_000_000
N_FEAT = 10
N_CORES = 8
P = 128
G = 64            # row groups (rows per moving column)
NJ = 5            # feature pairs

F_TOTAL = 7813    # moving columns per core
R = G * F_TOTAL   # 500_032 rows per core
T_PAD = R * N_CORES  # 4_000_256

# PSUM-bank-sized tiles (512 f32 columns).  Small first tile: compute
# starts sooner (shorter first DMA).
TILE_F = [133] + [512] * 15
assert sum(TILE_F) == F_TOTAL

DT = mybir.dt.float16
DT32 = mybir.dt.float32

_CACHE = {}


def _build():
    if "nc" in _CACHE:
        return _CACHE["nc"]
    nc = bacc.Bacc("TRN2", target_bir_lowering=False, debug=False,
                   num_devices=N_CORES)
    x_d = nc.dram_tensor("x", [P, NJ * F_TOTAL], DT, kind="ExternalInput")
    w_d = nc.dram_tensor("w", [P, NJ * P], DT, kind="ExternalInput")
    o_d = nc.dram_tensor("out", [P, F_TOTAL], DT, kind="ExternalOutput")

    x3 = x_d.ap().rearrange("p (j f) -> p j f", j=NJ)
    o2 = o_d.ap()

    with tile.TileContext(nc) as tc, ExitStack() as ctx:
        consts = ctx.enter_context(tc.tile_pool(name="consts", bufs=1))
        xpool = ctx.enter_context(tc.tile_pool(name="xp", bufs=3))
        opool = ctx.enter_context(tc.tile_pool(name="op", bufs=3))
        psum = ctx.enter_context(tc.tile_pool(name="ps", bufs=3, space="PSUM"))

        # weights via the SWDGE queue so the first x tile owns the Sync ring
        w_sb = consts.tile([P, NJ * P], DT)
        nc.gpsimd.dma_start(w_sb[:], w_d.ap())

        f0 = 0
        for i, FT in enumerate(TILE_F):
            x_sb = xpool.tile([P, NJ * FT], DT)
            nc.sync.dma_start(
                x_sb[:].rearrange("p (j f) -> p j f", j=NJ),
                x3[:, :, f0:f0 + FT],
            )
            ps = psum.tile([P, FT], DT32, name=f"ps_{i}", tag="ps")
            for j in range(NJ):
                nc.tensor.matmul(
                    ps[:],
                    w_sb[:, j * P:(j + 1) * P],
                    x_sb[:, j * FT:(j + 1) * FT],
                    start=(j == 0), stop=(j == NJ - 1),
                )
            o_sb = opool.tile([P, FT], DT)
            nc.scalar.copy(o_sb[:], ps[:])
            # SWDGE (gpsimd) store: keeps the Sync queue free to prefetch x
            nc.gpsimd.dma_start(o2[:, f0:f0 + FT], o_sb[:])
            f0 += FT

    nc.compile()
    _CACHE["nc"] = nc
    return nc


def _host_m(psi_real, psi_imag, A_real, A_imag):
    """M_r in float64: the coefficient matrix multiplying x_real."""
    pr = psi_real.astype(np.float64)
    pi = psi_imag.astype(np.float64)
    Ar = A_real.astype(np.float64)
    Ai = A_imag.astype(np.float64)

    def mat(p1, A, p2):
        return np.einsum("i,kija,j->ka", p1, A, p2)

    M = (mat(pr, Ar, pr) - mat(pi, Ai, pr)
         - mat(pr, Ar, pi) + mat(pi, Ai, pi))
    return M  # [2, 10] f64


def kernel(x, psi_real, psi_imag, A_real, A_imag, _trace=False):
    M = _host_m(psi_real, psi_imag, A_real, A_imag)

    # device layout: [core, p=2g+c, j, f] = x[core*R + f*64 + g, 2j+c]
    xq = np.zeros((T_PAD, N_FEAT), dtype=np.float16)
    xq[:T] = x
    X = (xq.reshape(N_CORES, F_TOTAL, G, NJ, 2)
         .transpose(0, 2, 4, 3, 1)
         .reshape(N_CORES, P, NJ * F_TOTAL))

    # W_j[2g+c, j*128 + 2g+k] = M[k, 2j+c]
    W = np.zeros((P, NJ * P), dtype=np.float16)
    g = np.arange(G)
    for j in range(NJ):
        for c in range(2):
            for k in range(2):
                W[2 * g + c, j * P + 2 * g + k] = np.float16(M[k, 2 * j + c])

    nc = _build()
    in_maps = [{"x": X[c], "w": W} for c in range(N_CORES)]
    res = run_bass_kernel_spmd(nc, in_maps, core_ids=list(range(N_CORES)),
                               trace=_trace)
    # out_dev[core, 2g+k, f] -> out[core*R + f*64 + g, k]
    O = np.stack([res.results[c]["out"] for c in range(N_CORES)])
    out = (O.reshape(N_CORES, G, 2, F_TOTAL)
           .transpose(0, 3, 1, 2)
           .reshape(T_PAD, 2)[:T]
           .astype(np.float32))
    if _trace:
        kernel.last_results = res
    return out


# revision 4
# speedup vs baseline: 1.1645x; 1.1645x over previous
"""Trainium2 Bass kernel for nn_ComplexNet: out = x @ M_r.T

Reference math: x_imag = 0, so only M_r (the real coefficient matrix,
[2, 10], built from psi/A via a tiny einsum) matters:
    out[t, k] = sum_a x[t, a] * M_r[k, a]

The problem is pure memory streaming; the only lever that matters is HBM
bytes moved.  The rel-err budget (2e-2) is ~30x looser than fp16
round-off, so the host stages x in fp16 (and reads the result back in
fp16): 12.3 MB/core instead of 24.6 MB/core -> ~34 us DMA floor at the
~360 GB/s per-core bus, vs 68 us for f32.

Layout (chosen so the device does zero data rearrangement):
  - Host casts + transposes x into a per-core fp16 tensor where
    partition p = 2g+c (g = row-group 0..63, c = pair lane) and column
    (j, f) holds x[row = f*64 + g, feature = 2j+c].
  - Columns are packed in 5 growing CHUNKS (133|512|1024|2048|4096 cols,
    each [128, 5, F_ch] j-major).  One DMA per chunk: each partition's
    data is a single contiguous 1.3-41 KB run -> single large
    descriptors at full bus rate (1 KB descriptors measured ~231 GB/s;
    >=4 KB is needed to saturate).  Growing sizes keep the pipeline
    head short while the tail runs at max descriptor size.
  - Stationary weights W_j [128, 128] (2x2 diagonal blocks):
    W_j[2g+c, 2g+k] = M[k, 2j+c].  Per 512-col PSUM tile, 5 fp16
    matmuls accumulate
      psum[2g+k, f] = sum_{j,c} x[f*64+g, 2j+c] * M[k, 2j+c]
    i.e. 12.8 rows/cycle; whole chunks resident in SBUF keep the PE
    busy back-to-back so its DVFS ramps to the warm 2.4 GHz clock.
  - ACT evicts each PSUM bank to a per-chunk fp16 staging tile; the
    otherwise-idle DVE HWDGE queue stores ~1024-col batches (2 KB
    descriptors), keeping the Sync queue free for input prefetch.

kernel(**inputs) takes the FULL unsharded inputs, returns the FULL
[4_000_000, 2] float32 output.
"""

import sys

import numpy as np

if "/opt/trn_rl_repo" not in sys.path:
    sys.path.insert(0, "/opt/trn_rl_repo")

from contextlib import ExitStack

import concourse.bacc as bacc
import concourse.tile as tile
from concourse import mybir
from concourse.bass_utils import run_bass_kernel_spmd

T = 4_000_000
N_FEAT = 10
N_CORES = 8
P = 128
G = 64            # row groups (rows per moving column)
NJ = 5            # feature pairs

F_TOTAL = 7813    # moving columns per core
R = G * F_TOTAL   # 500_032 rows per core
T_PAD = R * N_CORES  # 4_000_256

# PSUM-bank-sized tiles (<=512 f32 columns), grouped into chunks that
# share one input DMA.  Small head tiles start compute early.
CHUNKS = [[133], [512], [512, 512], [512] * 4, [512] * 8]
F_CH = [sum(c) for c in CHUNKS]
assert sum(F_CH) == F_TOTAL
# out-DMA batches: tiles per store within each chunk
OUT_BATCH = 2

DT = mybir.dt.float16
DT32 = mybir.dt.float32

_CACHE = {}


def _build():
    if "nc" in _CACHE:
        return _CACHE["nc"]
    nc = bacc.Bacc("TRN2", target_bir_lowering=False, debug=False,
                   num_devices=N_CORES)
    x_d = nc.dram_tensor("x", [P, NJ * F_TOTAL], DT, kind="ExternalInput")
    w_d = nc.dram_tensor("w", [P, NJ * P], DT, kind="ExternalInput")
    o_d = nc.dram_tensor("out", [P, F_TOTAL], DT, kind="ExternalOutput")

    x2 = x_d.ap()   # [p, chunk-packed cols]
    o2 = o_d.ap()   # [p, f] (f-major, global col order)

    with tile.TileContext(nc) as tc, ExitStack() as ctx:
        consts = ctx.enter_context(tc.tile_pool(name="consts", bufs=1))
        xpool = ctx.enter_context(tc.tile_pool(name="xp", bufs=2))
        opool = ctx.enter_context(tc.tile_pool(name="op", bufs=2))
        psum = ctx.enter_context(tc.tile_pool(name="ps", bufs=8, space="PSUM"))

        # weights via the SWDGE queue so the first x chunk owns the Sync ring
        w_sb = consts.tile([P, NJ * P], DT)
        nc.gpsimd.dma_start(w_sb[:], w_d.ap())

        g_off = 0   # global col offset (output order)
        x_off = 0   # col offset into the chunk-packed x dram tensor
        psum_i = 0
        for ci, tiles in enumerate(CHUNKS):
            FC = F_CH[ci]
            x_sb = xpool.tile([P, NJ * FC], DT, name=f"x_{ci}")
            nc.sync.dma_start(x_sb[:], x2[:, x_off:x_off + NJ * FC])
            o_sb = opool.tile([P, FC], DT, name=f"o_{ci}")

            b_off = 0
            batch = []
            for bi, FT in enumerate(tiles):
                ps = psum.tile([P, FT], DT32, name=f"ps_{psum_i}", tag="ps")
                psum_i += 1
                for j in range(NJ):
                    nc.tensor.matmul(
                        ps[:],
                        w_sb[:, j * P:(j + 1) * P],
                        x_sb[:, j * FC + b_off:j * FC + b_off + FT],
                        start=(j == 0), stop=(j == NJ - 1),
                    )
                nc.scalar.copy(o_sb[:, b_off:b_off + FT], ps[:])
                batch.append((b_off, FT))
                b_off += FT
                if len(batch) == OUT_BATCH or bi == len(tiles) - 1:
                    lo = batch[0][0]
                    ln = b_off - lo
                    # ACT HWDGE queue: keeps Sync free for prefetch (DVE
                    # has no DGE; gpsimd SWDGE is slow per descriptor)
                    nc.scalar.dma_start(
                        o2[:, g_off + lo:g_off + lo + ln],
                        o_sb[:, lo:lo + ln],
                    )
                    batch = []
            g_off += FC
            x_off += NJ * FC

    nc.compile()
    _CACHE["nc"] = nc
    return nc


def _host_m(psi_real, psi_imag, A_real, A_imag):
    """M_r in float64: the coefficient matrix multiplying x_real."""
    pr = psi_real.astype(np.float64)
    pi = psi_imag.astype(np.float64)
    Ar = A_real.astype(np.float64)
    Ai = A_imag.astype(np.float64)

    def mat(p1, A, p2):
        return np.einsum("i,kija,j->ka", p1, A, p2)

    M = (mat(pr, Ar, pr) - mat(pi, Ai, pr)
         - mat(pr, Ar, pi) + mat(pi, Ai, pi))
    return M  # [2, 10] f64


def kernel(x, psi_real, psi_imag, A_real, A_imag, _trace=False):
    M = _host_m(psi_real, psi_imag, A_real, A_imag)

    # logical layout: Y[core, p=2g+c, j, f] = x[core*R + f*64 + g, 2j+c]
    xq = np.zeros((T_PAD, N_FEAT), dtype=np.float16)
    xq[:T] = x
    Y = (xq.reshape(N_CORES, F_TOTAL, G, NJ, 2)
         .transpose(0, 2, 4, 3, 1)
         .reshape(N_CORES, P, NJ, F_TOTAL))
    # pack into per-chunk [p, j-major] blocks
    parts = []
    off = 0
    for FC in F_CH:
        parts.append(Y[:, :, :, off:off + FC].reshape(N_CORES, P, NJ * FC))
        off += FC
    X = np.ascontiguousarray(np.concatenate(parts, axis=2))

    # W_j[2g+c, j*128 + 2g+k] = M[k, 2j+c]
    W = np.zeros((P, NJ * P), dtype=np.float16)
    g = np.arange(G)
    for j in range(NJ):
        for c in range(2):
            for k in range(2):
                W[2 * g + c, j * P + 2 * g + k] = np.float16(M[k, 2 * j + c])

    nc = _build()
    in_maps = [{"x": X[c], "w": W} for c in range(N_CORES)]
    res = run_bass_kernel_spmd(nc, in_maps, core_ids=list(range(N_CORES)),
                               trace=_trace)
    # out_dev[core, 2g+k, f] -> out[core*R + f*64 + g, k]
    O = np.stack([res.results[c]["out"] for c in range(N_CORES)])
    out = (O.reshape(N_CORES, G, 2, F_TOTAL)
           .transpose(0, 3, 1, 2)
           .reshape(T_PAD, 2)[:T]
           .astype(np.float32))
    if _trace:
        kernel.last_results = res
    return out


# revision 7
# speedup vs baseline: 1.2930x; 1.1104x over previous
"""Trainium2 Bass kernel for nn_ComplexNet: out = x @ M_r.T

Reference math: x_imag = 0, so only M_r (the real coefficient matrix,
[2, 10], built from psi/A via a tiny einsum) matters:
    out[t, k] = sum_a x[t, a] * M_r[k, a]

The problem is pure memory streaming; the only lever that matters is HBM
bytes moved.  The rel-err budget (2e-2) is ~30x looser than fp16
round-off, so the host stages x in fp16 (and reads the result back in
fp16): 12.3 MB/core instead of 24.6 MB/core -> ~34 us DMA floor at the
~360 GB/s per-core bus, vs 68 us for f32.

Layout (chosen so the device does zero data rearrangement):
  - Host casts + transposes x into a per-core fp16 tensor where
    partition p = 2g+c (g = row-group 0..63, c = pair lane) and column
    (j, f) holds x[row = f*64 + g, feature = 2j+c].
  - Columns are packed in 5 growing CHUNKS (133|512|1024|2048|4096 cols,
    each [128, 5, F_ch] j-major).  One DMA per chunk: each partition's
    data is a single contiguous 1.3-41 KB run -> single large
    descriptors at full bus rate (1 KB descriptors measured ~231 GB/s;
    >=4 KB is needed to saturate).  Growing sizes keep the pipeline
    head short while the tail runs at max descriptor size.
  - Stationary weights W_j [128, 128] (2x2 diagonal blocks):
    W_j[2g+c, 2g+k] = M[k, 2j+c].  Per 512-col PSUM tile, 5 fp16
    matmuls accumulate
      psum[2g+k, f] = sum_{j,c} x[f*64+g, 2j+c] * M[k, 2j+c]
    i.e. 12.8 rows/cycle; whole chunks resident in SBUF keep the PE
    busy back-to-back so its DVFS ramps to the warm 2.4 GHz clock.
  - ACT evicts each PSUM bank to a per-chunk fp16 staging tile; the
    otherwise-idle DVE HWDGE queue stores ~1024-col batches (2 KB
    descriptors), keeping the Sync queue free for input prefetch.

kernel(**inputs) takes the FULL unsharded inputs, returns the FULL
[4_000_000, 2] float32 output.
"""

import sys

import numpy as np

if "/opt/trn_rl_repo" not in sys.path:
    sys.path.insert(0, "/opt/trn_rl_repo")

from contextlib import ExitStack

import concourse.bacc as bacc
import concourse.tile as tile
from concourse import mybir
from concourse.bass_utils import run_bass_kernel_spmd

T = 4_000_000
N_FEAT = 10
N_CORES = 8
P = 128
G = 64            # row groups (rows per moving column)
NJ = 5            # feature pairs

F_TOTAL = 7813    # moving columns per core
R = G * F_TOTAL   # 500_032 rows per core
T_PAD = R * N_CORES  # 4_000_256

# PSUM-bank-sized tiles (<=512 f32 columns), grouped into chunks that
# share one input DMA.  Small head tiles start compute early; 2-tile
# steady-state chunks (10 KB descriptors) keep the PE at most one
# 3.6 us transfer behind the input stream.
CHUNKS = [[133], [512]] + [[512, 512]] * 7
F_CH = [sum(c) for c in CHUNKS]
assert sum(F_CH) == F_TOTAL
# out-DMA batches: tiles per store within each chunk
OUT_BATCH = 2

DT = mybir.dt.float16
DT32 = mybir.dt.float32

_CACHE = {}


def _build():
    if "nc" in _CACHE:
        return _CACHE["nc"]
    nc = bacc.Bacc("TRN2", target_bir_lowering=False, debug=False,
                   num_devices=N_CORES)
    x_d = nc.dram_tensor("x", [P, NJ * F_TOTAL], DT, kind="ExternalInput")
    w_d = nc.dram_tensor("w", [P, NJ * P], DT, kind="ExternalInput")
    o_d = nc.dram_tensor("out", [P, F_TOTAL], DT, kind="ExternalOutput")

    x2 = x_d.ap()   # [p, chunk-packed cols]
    o2 = o_d.ap()   # [p, f] (f-major, global col order)

    with tile.TileContext(nc) as tc, ExitStack() as ctx:
        consts = ctx.enter_context(tc.tile_pool(name="consts", bufs=1))
        xpool = ctx.enter_context(tc.tile_pool(name="xp", bufs=2))
        opool = ctx.enter_context(tc.tile_pool(name="op", bufs=2))
        psum = ctx.enter_context(tc.tile_pool(name="ps", bufs=8, space="PSUM"))

        # weights first on the Sync queue: tiny (0.5 us) and the PE can't
        # start without them (on gpsimd SWDGE it didn't land until ~16 us)
        w_sb = consts.tile([P, NJ * P], DT)
        nc.sync.dma_start(w_sb[:], w_d.ap())

        g_off = 0   # global col offset (output order)
        x_off = 0   # col offset into the chunk-packed x dram tensor
        psum_i = 0
        for ci, tiles in enumerate(CHUNKS):
            FC = F_CH[ci]
            x_sb = xpool.tile([P, NJ * FC], DT, name=f"x_{ci}")
            nc.sync.dma_start(x_sb[:], x2[:, x_off:x_off + NJ * FC])
            o_sb = opool.tile([P, FC], DT, name=f"o_{ci}")

            # j-outer: consecutive matmuls share one stationary (gives the
            # compiler a shot at skipping redundant LDWEIGHTS)
            offs = []
            b_off = 0
            pss = []
            for FT in tiles:
                offs.append((b_off, FT))
                pss.append(psum.tile([P, FT], DT32, name=f"ps_{psum_i}", tag="ps"))
                psum_i += 1
                b_off += FT
            for j in range(NJ):
                for (bo, FT), ps in zip(offs, pss):
                    nc.tensor.matmul(
                        ps[:],
                        w_sb[:, j * P:(j + 1) * P],
                        x_sb[:, j * FC + bo:j * FC + bo + FT],
                        start=(j == 0), stop=(j == NJ - 1),
                    )
            for (bo, FT), ps in zip(offs, pss):
                nc.scalar.copy(o_sb[:, bo:bo + FT], ps[:])
            # ACT HWDGE queue: keeps Sync free for prefetch (DVE has no
            # DGE; gpsimd SWDGE is slow per descriptor)
            nc.scalar.dma_start(o2[:, g_off:g_off + FC], o_sb[:])
            g_off += FC
            x_off += NJ * FC

    nc.compile()
    _CACHE["nc"] = nc
    return nc


def _host_m(psi_real, psi_imag, A_real, A_imag):
    """M_r in float64: the coefficient matrix multiplying x_real."""
    pr = psi_real.astype(np.float64)
    pi = psi_imag.astype(np.float64)
    Ar = A_real.astype(np.float64)
    Ai = A_imag.astype(np.float64)

    def mat(p1, A, p2):
        return np.einsum("i,kija,j->ka", p1, A, p2)

    M = (mat(pr, Ar, pr) - mat(pi, Ai, pr)
         - mat(pr, Ar, pi) + mat(pi, Ai, pi))
    return M  # [2, 10] f64


def kernel(x, psi_real, psi_imag, A_real, A_imag, _trace=False):
    M = _host_m(psi_real, psi_imag, A_real, A_imag)

    # logical layout: Y[core, p=2g+c, j, f] = x[core*R + f*64 + g, 2j+c]
    xq = np.zeros((T_PAD, N_FEAT), dtype=np.float16)
    xq[:T] = x
    Y = (xq.reshape(N_CORES, F_TOTAL, G, NJ, 2)
         .transpose(0, 2, 4, 3, 1)
         .reshape(N_CORES, P, NJ, F_TOTAL))
    # pack into per-chunk [p, j-major] blocks
    parts = []
    off = 0
    for FC in F_CH:
        parts.append(Y[:, :, :, off:off + FC].reshape(N_CORES, P, NJ * FC))
        off += FC
    X = np.ascontiguousarray(np.concatenate(parts, axis=2))

    # W_j[2g+c, j*128 + 2g+k] = M[k, 2j+c]
    W = np.zeros((P, NJ * P), dtype=np.float16)
    g = np.arange(G)
    for j in range(NJ):
        for c in range(2):
            for k in range(2):
                W[2 * g + c, j * P + 2 * g + k] = np.float16(M[k, 2 * j + c])

    nc = _build()
    in_maps = [{"x": X[c], "w": W} for c in range(N_CORES)]
    res = run_bass_kernel_spmd(nc, in_maps, core_ids=list(range(N_CORES)),
                               trace=_trace)
    # out_dev[core, 2g+k, f] -> out[core*R + f*64 + g, k]
    O = np.stack([res.results[c]["out"] for c in range(N_CORES)])
    out = (O.reshape(N_CORES, G, 2, F_TOTAL)
           .transpose(0, 3, 1, 2)
           .reshape(T_PAD, 2)[:T]
           .astype(np.float32))
    if _trace:
        kernel.last_results = res
    return out
